# revision 21
# baseline (speedup 1.0000x reference)
"""DNGPU cell (gated conv recurrence) for Trainium2, data-parallel over batch on 8 cores.

Problem: B=32, L=128, C=192, K=3; 128 sequential steps of
    reset = sigmoid(conv(mem, w_reset) + 0.5)
    gate  = sigmoid(conv(mem, w_gate) + 0.7)
    cand  = tanh(conv(reset*mem, w_cand))
    mem   = gate*shift_right(mem) + (1-gate)*cand

Per-core layout: state held in SBUF as [C partitions, token cols] where
token col = 4 + l*4 + b  (l-major, b-minor, 4 zero-pad cols each side).
Conv taps are column-shifted views, shift_right is the view shifted by
-4. C=192 splits into an A half (channels 0:128) and B half (128:192).
Matmuls run in float32r (full PE rate at N>=256).

v3 structure:
- reset+gate convs read the same input and are packed into ONE
  384-output conv = 3 output slots of 128 (S0=r[0:128], S1=g[0:128],
  S2=[r[128:192]; g[128:192]]): 15 matmuls, 3 sigmoids.
- The B-half dup lane is REVERSED (mB[64:128, c] = chan[c-4], written at
  cols 8:8+T; packed rg cinB weights are [tap1; tap0] reading cols
  4:4+T). This makes mB[:, 4:4+T] = [token l; token l-1] per partition
  half, so ONE fused Vector mul of sig2 (=[rB; gB]) by that view yields
  rmemB (rows 0:64) AND uB = gB*shift (rows 64:128) in a scratch sB.
- cand conv's B taps read sB[0:64] directly via column shifts
  (unpacked, 3x 64-contraction matmuls per slot): no rmem dup at all.
- B-half combine runs at partition base 64 (tanh crosses PSUM->base64,
  qB/subB have base-64-aligned inputs, subB writes back to base 0).
- No GpSimd compute (its TT ops are ~1.4us and stretch concurrent DVE
  ops 2-6x - measured); everything elementwise is Vector/Scalar.
- Critical-path ops (sig0, rmemA, tanhA, qA, subA) are split into
  256-col halves to cut latency; keep-warm dummy matmuls bridge the
  two unavoidable PE gaps so HAM holds the 2.4GHz p-state.
"""

import numpy as np
from contextlib import ExitStack

import concourse.bacc as bacc
import concourse.tile as tile
from concourse import mybir
from concourse.tile import add_dep_helper
from concourse.bass_utils import run_bass_kernel_spmd

B, L, C = 32, 128, 192
NCORES = 8
BLOC = B // NCORES          # 4 batches per core
TOK = BLOC * L              # 512 tokens per core
WPAD = TOK + 8              # 4 zero cols each side
STEPS = 128
HTOK = TOK // 2             # half-width for split critical-path ops

F32 = mybir.dt.float32
F32R = mybir.dt.float32r
AF = mybir.ActivationFunctionType
ALU = mybir.AluOpType

N_DUMMY_MID = 1             # bridge rg->cand rmemA latency
N_DUMMY_TAIL = 4            # bridge combine tail


def build(steps=STEPS):
    nc = bacc.Bacc("TRN2", target_bir_lowering=False, debug=False,
                   num_devices=NCORES)
    x_d = nc.dram_tensor("x", [BLOC, L, C], F32, kind="ExternalInput").ap()
    w_d = {}
    b_d = {}
    for cv, wn, bn in (("r", "w_reset", "b_reset"),
                       ("g", "w_gate", "b_gate"),
                       ("n", "w_cand", "b_cand")):
        w_d[cv] = nc.dram_tensor(wn, [3, C, C], F32, kind="ExternalInput").ap()
        b_d[cv] = nc.dram_tensor(bn, [C], F32, kind="ExternalInput").ap()
    id_d = nc.dram_tensor("ident", [128, 128], F32, kind="ExternalInput").ap()
    out_d = nc.dram_tensor("out", [BLOC, L, C], F32, kind="ExternalOutput").ap()

    with tile.TileContext(nc) as tc, ExitStack() as ctx:
        const = ctx.enter_context(tc.tile_pool(name="const", bufs=1))
        state = ctx.enter_context(tc.tile_pool(name="state", bufs=1))
        act = ctx.enter_context(tc.tile_pool(name="act", bufs=6))
        tmp = ctx.enter_context(tc.tile_pool(name="tmp", bufs=4))
        psum = ctx.enter_context(tc.tile_pool(name="psum", bufs=1, space="PSUM"))

        # --- constants ---------------------------------------------------
        # rg slots: S0 = r[:,0:128], S1 = g[:,0:128], S2 = [r|g][:,128:192]
        SLOT_PARTS = (
            (("r", 0, 128, 0, 128),),                            # S0
            (("g", 0, 128, 0, 128),),                            # S1
            (("r", 128, 64, 0, 64), ("g", 128, 64, 64, 64)))     # S2
        wA = {}    # (slot, k) -> [128, 128] cinA tap k
        wpB = {}   # slot -> [128, 128]: rows 0:64 = TAP1 chanB, 64:128 = TAP0
        w3B = {}   # slot -> [64, 128]   tap2 chanB
        for s in range(3):
            parts = SLOT_PARTS[s]
            for k in range(3):
                t = const.tile([128, 128], F32R, tag=f"wA{s}{k}", name=f"wA{s}{k}")
                for cv, o0, ol, d0, dl in parts:
                    nc.gpsimd.dma_start(t[:, d0:d0 + dl],
                                        w_d[cv][k, 0:128, o0:o0 + ol])
                wA[s, k] = t
            t = const.tile([128, 128], F32R, tag=f"wpB{s}", name=f"wpB{s}")
            for cv, o0, ol, d0, dl in parts:
                nc.gpsimd.dma_start(t[0:64, d0:d0 + dl],
                                    w_d[cv][1, 128:192, o0:o0 + ol])
                nc.gpsimd.dma_start(t[64:128, d0:d0 + dl],
                                    w_d[cv][0, 128:192, o0:o0 + ol])
            wpB[s] = t
            t = const.tile([64, 128], F32R, tag=f"w3B{s}", name=f"w3B{s}")
            for cv, o0, ol, d0, dl in parts:
                nc.gpsimd.dma_start(t[:, d0:d0 + dl],
                                    w_d[cv][2, 128:192, o0:o0 + ol])
            w3B[s] = t
        # cand slots: C0 = n[:,0:128], C1 = n[:,128:192]; cinB unpacked
        wAn = {}
        wBn = {}
        for co, (o0, ol) in enumerate(((0, 128), (128, 64))):
            for k in range(3):
                t = const.tile([128, ol], F32R, tag=f"wAn{co}{k}", name=f"wAn{co}{k}")
                nc.gpsimd.dma_start(t[:], w_d["n"][k, 0:128, o0:o0 + ol])
                wAn[co, k] = t
                t = const.tile([64, ol], F32R, tag=f"wBn{co}{k}", name=f"wBn{co}{k}")
                nc.gpsimd.dma_start(t[:], w_d["n"][k, 128:192, o0:o0 + ol])
                wBn[co, k] = t
        # biases per slot: [128,1]
        BIAS_PARTS = (
            (("r", 0, 128, 0),),
            (("g", 0, 128, 0),),
            (("r", 128, 64, 0), ("g", 128, 64, 64)))
        bias = {}
        for s in range(3):
            t = const.tile([128, 1], F32, tag=f"bias{s}", name=f"bias{s}")
            for cv, c0, cl, d0 in BIAS_PARTS[s]:
                nc.sync.dma_start(t[d0:d0 + cl, 0], b_d[cv][c0:c0 + cl])
            bias[s] = t
        ident = const.tile([128, 128], F32, tag="ident")
        nc.sync.dma_start(ident[:], id_d)
        identr = const.tile([128, 128], F32R, tag="identr")
        nc.gpsimd.dma_start(identr[:], id_d)

        # --- state tiles ---------------------------------------------------
        mem = {}
        for i in range(2):
            mem[i, 0] = state.tile([128, WPAD], F32R, tag=f"memA{i}", name=f"memA{i}")
            mem[i, 1] = state.tile([128, WPAD], F32R, tag=f"memB{i}", name=f"memB{i}")
        rmemA = state.tile([128, WPAD], F32R, tag="rmemA", name="rmemA")
        sB = state.tile([128, WPAD], F32R, tag="sB", name="sB")
        zf32 = state.tile([128, WPAD], F32, tag="zf32", name="zf32")
        nc.gpsimd.memset(zf32[:], 0.0)
        for t in list(mem.values()) + [rmemA, sB]:
            nc.vector.tensor_copy(t[:], zf32[:])

        # --- input transform: x[b,l,c] -> mem[0] = [c, 4+l*4+b] ------------
        for b in range(BLOC):
            xb = tmp.tile([L, C], F32, tag="xload")
            nc.sync.dma_start(xb[:], x_d[b])
            for ci, (c0, cl) in enumerate(((0, 128), (128, 64))):
                ps = psum.tile([cl, L], F32, tag=f"tp{ci}")
                nc.tensor.transpose(ps[:], xb[:, c0:c0 + cl], ident[:])
                dst = mem[0, ci][0:cl, 4 + b: 4 + b + 4 * L: 4]
                nc.vector.tensor_copy(dst, ps[:])

        # initial reversed dup: mB[64:128, c] = chan[c-4] (cols 8:8+T)
        nc.vector.tensor_copy(mem[0, 1][64:128, 8:8 + TOK],
                              mem[0, 1][0:64, 4:4 + TOK])

        # --- recurrence -----------------------------------------------------
        cur = 0
        for t in range(steps):
            mA = mem[cur, 0]
            mB = mem[cur, 1]
            nA = mem[1 - cur, 0]
            nB = mem[1 - cur, 1]

            # rg psum slots
            ps = [psum.tile([128, TOK], F32, tag=f"rg{s}", name=f"rg{s}")
                  for s in range(3)]

            # Matmul order tuned on the trace: slots close in urgency order
            # (s0 -> sigmoid/rmemA chain, s2 -> fusedB -> cand cinB, s1 ->
            # tail q). s0's cinB pair sits at matmul #8-9 - the earliest
            # position that still clears the previous step's B-tail (subB +
            # dupB land ~1.8us after restart). 128<->64 contraction switches
            # cost ~150-200ns each, so pairs are kept adjacent.
            def cinA(s, k):
                nc.tensor.matmul(ps[s][:], wA[s, k][:],
                                 mA[:, k * 4: k * 4 + TOK],
                                 start=(k == 0), stop=False)

            def cinB_pair(s):
                nc.tensor.matmul(ps[s][:], wpB[s][:], mB[:, 4:4 + TOK],
                                 start=False, stop=False)
                nc.tensor.matmul(ps[s][:], w3B[s][:], mB[0:64, 8:8 + TOK],
                                 start=False, stop=True)

            # all 9 cinA first - pulling s0's cinB pair earlier than matmul
            # #10 head-blocks the in-order PE queue on the previous step's
            # dupB (measured: +280us total)
            for s in (0, 2, 1):
                for k in range(3):
                    cinA(s, k)
            cinB_pair(0)
            nc.tensor.matmul(ps[2][:], wpB[2][:], mB[:, 4:4 + TOK],
                             start=False, stop=False)
            nc.tensor.matmul(ps[1][:], wpB[1][:], mB[:, 4:4 + TOK],
                             start=False, stop=False)
            nc.tensor.matmul(ps[2][:], w3B[2][:], mB[0:64, 8:8 + TOK],
                             start=False, stop=True)
            nc.tensor.matmul(ps[1][:], w3B[1][:], mB[0:64, 8:8 + TOK],
                             start=False, stop=True)

            # sigmoids in urgency order (s0 -> rmemA -> candA chain; s2 ->
            # fusedB -> cand cinB; s1 -> tail q); pin the queue order with
            # explicit edges so the backend scheduler can't reorder
            sig = [act.tile([128, TOK], F32R, tag=f"sig{s}", name=f"sig{s}")
                   for s in range(3)]
            s0i = nc.scalar.activation(sig[0][:], ps[0][:], AF.Sigmoid,
                                       bias=bias[0][:, 0:1])
            s2i = nc.scalar.activation(sig[2][:], ps[2][:], AF.Sigmoid,
                                       bias=bias[2][:, 0:1])
            add_dep_helper(s2i.ins, s0i.ins, sync=False,
                           reason="sig0 before sig2")
            s1i = nc.scalar.activation(sig[1][:], ps[1][:], AF.Sigmoid,
                                       bias=bias[1][:, 0:1])
            add_dep_helper(s1i.ins, s2i.ins, sync=False,
                           reason="sig2 before sig1")

            # rmemA; fused B mul gives rmemB (rows 0:64) + uB (64:128)
            nc.vector.tensor_mul(rmemA[:, 4:4 + TOK], sig[0][:],
                                 mA[:, 4:4 + TOK])
            nc.vector.tensor_mul(sB[:, 4:4 + TOK], sig[2][:], mB[:, 4:4 + TOK])
            # u = gate * shifted (A half)
            uA = tmp.tile([128, TOK], F32R, tag="uA", name="uA")
            nc.vector.tensor_mul(uA[:], sig[1][:], mA[:, 0:TOK])

            # keep-warm dummies bridge the rg->cand rmemA latency
            dummy = psum.tile([128, 384], F32, tag="dm", name=f"dm{t}")
            for dk in range(N_DUMMY_MID):
                nc.tensor.matmul(dummy[:], wAn[0, 0][:],
                                 mA[:, 0:384], start=True, stop=True)

            # cand conv: A slot first (drives the critical tail); cinB taps
            # read sB[0:64] (rmemB) directly via column shifts
            psn = [psum.tile([128, TOK], F32, tag="nA", name="psnA"),
                   psum.tile([64, TOK], F32, tag="nB", name="psnB")]
            # order: cinA0(128c), cinB0(64c), cinB1(64c), cinA1(128c) - candA
            # still closes at matmul 6/12 but only 2 contraction switches
            for k in range(3):
                nc.tensor.matmul(psn[0][:], wAn[0, k][:],
                                 rmemA[:, k * 4: k * 4 + TOK],
                                 start=(k == 0), stop=False)
            for k in range(3):
                nc.tensor.matmul(psn[0][:], wBn[0, k][:],
                                 sB[0:64, k * 4: k * 4 + TOK],
                                 start=False, stop=(k == 2))
            for k in range(3):
                nc.tensor.matmul(psn[1][:], wBn[1, k][:],
                                 sB[0:64, k * 4: k * 4 + TOK],
                                 start=(k == 0), stop=False)
            for k in range(3):
                nc.tensor.matmul(psn[1][:], wAn[1, k][:],
                                 rmemA[:, k * 4: k * 4 + TOK],
                                 start=False, stop=(k == 2))

            # A-half tail split into halves: tanh -> q -> sub per 256 cols
            candA = act.tile([128, TOK], F32R, tag="cdA", name="cdA")
            candB = act.tile([128, TOK], F32R, tag="cdB", name="cdB")
            qA = tmp.tile([128, TOK], F32R, tag="qA", name="qA")
            qB = tmp.tile([128, TOK], F32R, tag="qB", name="qB")
            for h0, h1 in ((0, HTOK), (HTOK, TOK)):
                nc.scalar.activation(candA[:, h0:h1], psn[0][:, h0:h1], AF.Tanh)
            # tanhB crosses PSUM base 0 -> SBUF base 64
            nc.scalar.activation(candB[64:128, :], psn[1][:], AF.Tanh)
            for h0, h1 in ((0, HTOK), (HTOK, TOK)):
                nc.vector.scalar_tensor_tensor(
                    qA[:, h0:h1], sig[1][:, h0:h1], 1.0, candA[:, h0:h1],
                    op0=ALU.subtract, op1=ALU.mult)

            # keep-warm dummies for the restart gap: reading qA pins them
            # after the tail starts (dep-free dummies get hoisted by the
            # scheduler into the mid-step slot, delaying the cand conv)
            for dk in range(N_DUMMY_TAIL):
                nc.tensor.matmul(dummy[:], wAn[0, 0][:],
                                 qA[:, 0:384], start=True, stop=True)
            for h0, h1 in ((0, HTOK), (HTOK, TOK)):
                nc.vector.tensor_sub(nA[:, 4 + h0:4 + h1], uA[:, h0:h1],
                                     qA[:, h0:h1])
            # B-half combine at partition base 64; subB writes base 0
            nc.vector.scalar_tensor_tensor(
                qB[64:128, :], sig[2][64:128, :], 1.0, candB[64:128, :],
                op0=ALU.subtract, op1=ALU.mult)
            nc.vector.tensor_sub(nB[0:64, 4:4 + TOK], sB[64:128, 4:4 + TOK],
                                 qB[64:128, :])
            nc.vector.tensor_copy(nB[64:128, 8:8 + TOK], nB[0:64, 4:4 + TOK])

            cur = 1 - cur

        # --- output transform: mem[cur] -> out[b,l,c] -----------------------
        for b in range(BLOC):
            osb = tmp.tile([L, C], F32, tag="oload")
            for ci, (c0, cl) in enumerate(((0, 128), (128, 64))):
                ps = psum.tile([L, cl], F32R, tag=f"tp{ci}")
                nc.tensor.transpose(ps[:], mem[cur, ci][0:cl, 4 + b: 4 + b + 4 * L: 4],
                                    identr[0:cl, 0:cl])
                nc.vector.tensor_copy(osb[:, c0:c0 + cl], ps[:])
            nc.sync.dma_start(out_d[b], osb[:])

    nc.compile()
    return nc


_built = {}


def _get(steps=STEPS):
    if steps not in _built:
        _built[steps] = build(steps)
    return _built[steps]


def kernel(x, w_reset, b_reset, w_gate, b_gate, w_cand, b_cand, steps=STEPS,
           trace=False):
    nc = _get(steps)
    ident = np.eye(128, dtype=np.float32)
    base = {"w_reset": np.asarray(w_reset, np.float32),
            "b_reset": np.asarray(b_reset, np.float32),
            "w_gate": np.asarray(w_gate, np.float32),
            "b_gate": np.asarray(b_gate, np.float32),
            "w_cand": np.asarray(w_cand, np.float32),
            "b_cand": np.asarray(b_cand, np.float32),
            "ident": ident}
    x = np.asarray(x, np.float32)
    in_maps = [dict(base, x=np.ascontiguousarray(x[i * BLOC:(i + 1) * BLOC]))
               for i in range(NCORES)]
    res = run_bass_kernel_spmd(nc, in_maps, core_ids=list(range(NCORES)),
                               trace=trace)
    out = np.concatenate([res.results[i]["out"] for i in range(NCORES)], axis=0)
    if trace:
        return out, res
    return out


if __name__ == "__main__":
    rng = np.random.default_rng(0)
    scale = 1.0 / np.sqrt(3 * C)
    ins = {
        "x": rng.standard_normal((B, L, C), dtype=np.float32),
        "w_reset": (rng.standard_normal((3, C, C)) * scale).astype(np.float32),
        "b_reset": np.full(C, 0.5, np.float32),
        "w_gate": (rng.standard_normal((3, C, C)) * scale).astype(np.float32),
        "b_gate": np.full(C, 0.7, np.float32),
        "w_cand": (rng.standard_normal((3, C, C)) * scale).astype(np.float32),
        "b_cand": np.zeros(C, np.float32),
    }
    out = kernel(**ins, steps=2)
    print("smoke ok", out.shape, out.dtype)


# revision 23
# speedup vs baseline: 1.0180x; 1.0180x over previous
"""DNGPU cell (gated conv recurrence) for Trainium2, data-parallel over batch on 8 cores.

Problem: B=32, L=128, C=192, K=3; 128 sequential steps of
    reset = sigmoid(conv(mem, w_reset) + 0.5)
    gate  = sigmoid(conv(mem, w_gate) + 0.7)
    cand  = tanh(conv(reset*mem, w_cand))
    mem   = gate*shift_right(mem) + (1-gate)*cand

Per-core layout: state held in SBUF as [C partitions, token cols] where
token col = 4 + l*4 + b  (l-major, b-minor, 4 zero-pad cols each side).
Conv taps are column-shifted views, shift_right is the view shifted by
-4. C=192 splits into an A half (channels 0:128) and B half (128:192).
Matmuls run in float32r (full PE rate at N>=256).

v3 structure:
- reset+gate convs read the same input and are packed into ONE
  384-output conv = 3 output slots of 128 (S0=r[0:128], S1=g[0:128],
  S2=[r[128:192]; g[128:192]]): 15 matmuls, 3 sigmoids.
- The B-half dup lane is REVERSED (mB[64:128, c] = chan[c-4], written at
  cols 8:8+T; packed rg cinB weights are [tap1; tap0] reading cols
  4:4+T). This makes mB[:, 4:4+T] = [token l; token l-1] per partition
  half, so ONE fused Vector mul of sig2 (=[rB; gB]) by that view yields
  rmemB (rows 0:64) AND uB = gB*shift (rows 64:128) in a scratch sB.
- cand conv's B taps read sB[0:64] directly via column shifts
  (unpacked, 3x 64-contraction matmuls per slot): no rmem dup at all.
- B-half combine runs at partition base 64 (tanh crosses PSUM->base64,
  qB/subB have base-64-aligned inputs, subB writes back to base 0).
- No GpSimd compute (its TT ops are ~1.4us and stretch concurrent DVE
  ops 2-6x - measured); everything elementwise is Vector/Scalar.
- Critical-path ops (sig0, rmemA, tanhA, qA, subA) are split into
  256-col halves to cut latency; keep-warm dummy matmuls bridge the
  two unavoidable PE gaps so HAM holds the 2.4GHz p-state.
"""

import numpy as np
from contextlib import ExitStack

import concourse.bacc as bacc
import concourse.tile as tile
from concourse import mybir
from concourse.tile import add_dep_helper
from concourse.bass_utils import run_bass_kernel_spmd

B, L, C = 32, 128, 192
NCORES = 8
BLOC = B // NCORES          # 4 batches per core
TOK = BLOC * L              # 512 tokens per core
WPAD = TOK + 8              # 4 zero cols each side
STEPS = 128
HTOK = TOK // 2             # half-width for split critical-path ops

F32 = mybir.dt.float32
F32R = mybir.dt.float32r
AF = mybir.ActivationFunctionType
ALU = mybir.AluOpType

N_DUMMY_MID = 2             # bridge rg->cand rmemA latency
N_DUMMY_TAIL = 3            # bridge combine tail


def build(steps=STEPS):
    nc = bacc.Bacc("TRN2", target_bir_lowering=False, debug=False,
                   num_devices=NCORES)
    x_d = nc.dram_tensor("x", [BLOC, L, C], F32, kind="ExternalInput").ap()
    w_d = {}
    b_d = {}
    for cv, wn, bn in (("r", "w_reset", "b_reset"),
                       ("g", "w_gate", "b_gate"),
                       ("n", "w_cand", "b_cand")):
        w_d[cv] = nc.dram_tensor(wn, [3, C, C], F32, kind="ExternalInput").ap()
        b_d[cv] = nc.dram_tensor(bn, [C], F32, kind="ExternalInput").ap()
    id_d = nc.dram_tensor("ident", [128, 128], F32, kind="ExternalInput").ap()
    out_d = nc.dram_tensor("out", [BLOC, L, C], F32, kind="ExternalOutput").ap()

    with tile.TileContext(nc) as tc, ExitStack() as ctx:
        const = ctx.enter_context(tc.tile_pool(name="const", bufs=1))
        state = ctx.enter_context(tc.tile_pool(name="state", bufs=1))
        act = ctx.enter_context(tc.tile_pool(name="act", bufs=6))
        tmp = ctx.enter_context(tc.tile_pool(name="tmp", bufs=4))
        psum = ctx.enter_context(tc.tile_pool(name="psum", bufs=1, space="PSUM"))

        # --- constants ---------------------------------------------------
        # rg slots: S0 = r[:,0:128], S1 = g[:,0:128], S2 = [r|g][:,128:192]
        SLOT_PARTS = (
            (("r", 0, 128, 0, 128),),                            # S0
            (("g", 0, 128, 0, 128),),                            # S1
            (("r", 128, 64, 0, 64), ("g", 128, 64, 64, 64)))     # S2
        wA = {}    # (slot, k) -> [128, 128] cinA tap k
        wpB = {}   # slot -> [128, 128]: rows 0:64 = TAP1 chanB, 64:128 = TAP0
        w3B = {}   # slot -> [64, 128]   tap2 chanB
        for s in range(3):
            parts = SLOT_PARTS[s]
            for k in range(3):
                t = const.tile([128, 128], F32R, tag=f"wA{s}{k}", name=f"wA{s}{k}")
                for cv, o0, ol, d0, dl in parts:
                    nc.gpsimd.dma_start(t[:, d0:d0 + dl],
                                        w_d[cv][k, 0:128, o0:o0 + ol])
                wA[s, k] = t
            t = const.tile([128, 128], F32R, tag=f"wpB{s}", name=f"wpB{s}")
            for cv, o0, ol, d0, dl in parts:
                nc.gpsimd.dma_start(t[0:64, d0:d0 + dl],
                                    w_d[cv][1, 128:192, o0:o0 + ol])
                nc.gpsimd.dma_start(t[64:128, d0:d0 + dl],
                                    w_d[cv][0, 128:192, o0:o0 + ol])
            wpB[s] = t
            t = const.tile([64, 128], F32R, tag=f"w3B{s}", name=f"w3B{s}")
            for cv, o0, ol, d0, dl in parts:
                nc.gpsimd.dma_start(t[:, d0:d0 + dl],
                                    w_d[cv][2, 128:192, o0:o0 + ol])
            w3B[s] = t
        # cand slots: C0 = n[:,0:128], C1 = n[:,128:192]; cinB unpacked
        wAn = {}
        wBn = {}
        for co, (o0, ol) in enumerate(((0, 128), (128, 64))):
            for k in range(3):
                t = const.tile([128, ol], F32R, tag=f"wAn{co}{k}", name=f"wAn{co}{k}")
                nc.gpsimd.dma_start(t[:], w_d["n"][k, 0:128, o0:o0 + ol])
                wAn[co, k] = t
                t = const.tile([64, ol], F32R, tag=f"wBn{co}{k}", name=f"wBn{co}{k}")
                nc.gpsimd.dma_start(t[:], w_d["n"][k, 128:192, o0:o0 + ol])
                wBn[co, k] = t
        # biases per slot: [128,1]
        BIAS_PARTS = (
            (("r", 0, 128, 0),),
            (("g", 0, 128, 0),),
            (("r", 128, 64, 0), ("g", 128, 64, 64)))
        bias = {}
        for s in range(3):
            t = const.tile([128, 1], F32, tag=f"bias{s}", name=f"bias{s}")
            for cv, c0, cl, d0 in BIAS_PARTS[s]:
                nc.sync.dma_start(t[d0:d0 + cl, 0], b_d[cv][c0:c0 + cl])
            bias[s] = t
        ident = const.tile([128, 128], F32, tag="ident")
        nc.sync.dma_start(ident[:], id_d)
        identr = const.tile([128, 128], F32R, tag="identr")
        nc.gpsimd.dma_start(identr[:], id_d)

        # --- state tiles ---------------------------------------------------
        mem = {}
        for i in range(2):
            mem[i, 0] = state.tile([128, WPAD], F32R, tag=f"memA{i}", name=f"memA{i}")
            mem[i, 1] = state.tile([128, WPAD], F32R, tag=f"memB{i}", name=f"memB{i}")
        rmemA = state.tile([128, WPAD], F32R, tag="rmemA", name="rmemA")
        sB = state.tile([128, WPAD], F32R, tag="sB", name="sB")
        zf32 = state.tile([128, WPAD], F32, tag="zf32", name="zf32")
        nc.gpsimd.memset(zf32[:], 0.0)
        for t in list(mem.values()) + [rmemA, sB]:
            nc.vector.tensor_copy(t[:], zf32[:])

        # --- input transform: x[b,l,c] -> mem[0] = [c, 4+l*4+b] ------------
        for b in range(BLOC):
            xb = tmp.tile([L, C], F32, tag="xload")
            nc.sync.dma_start(xb[:], x_d[b])
            for ci, (c0, cl) in enumerate(((0, 128), (128, 64))):
                ps = psum.tile([cl, L], F32, tag=f"tp{ci}")
                nc.tensor.transpose(ps[:], xb[:, c0:c0 + cl], ident[:])
                dst = mem[0, ci][0:cl, 4 + b: 4 + b + 4 * L: 4]
                nc.vector.tensor_copy(dst, ps[:])

        # initial reversed dup: mB[64:128, c] = chan[c-4] (cols 8:8+T)
        nc.vector.tensor_copy(mem[0, 1][64:128, 8:8 + TOK],
                              mem[0, 1][0:64, 4:4 + TOK])

        # --- recurrence -----------------------------------------------------
        cur = 0
        for t in range(steps):
            mA = mem[cur, 0]
            mB = mem[cur, 1]
            nA = mem[1 - cur, 0]
            nB = mem[1 - cur, 1]

            # rg psum slots
            ps = [psum.tile([128, TOK], F32, tag=f"rg{s}", name=f"rg{s}")
                  for s in range(3)]

            # Matmul order tuned on the trace: slots close in urgency order
            # (s0 -> sigmoid/rmemA chain, s2 -> fusedB -> cand cinB, s1 ->
            # tail q). s0's cinB pair sits at matmul #8-9 - the earliest
            # position that still clears the previous step's B-tail (subB +
            # dupB land ~1.8us after restart). 128<->64 contraction switches
            # cost ~150-200ns each, so pairs are kept adjacent.
            def cinA(s, k):
                nc.tensor.matmul(ps[s][:], wA[s, k][:],
                                 mA[:, k * 4: k * 4 + TOK],
                                 start=(k == 0), stop=False)

            def cinB_pair(s):
                nc.tensor.matmul(ps[s][:], wpB[s][:], mB[:, 4:4 + TOK],
                                 start=False, stop=False)
                nc.tensor.matmul(ps[s][:], w3B[s][:], mB[0:64, 8:8 + TOK],
                                 start=False, stop=True)

            # all 9 cinA first - pulling s0's cinB pair earlier than matmul
            # #10 head-blocks the in-order PE queue on the previous step's
            # dupB (measured: +280us total)
            for s in (0, 2, 1):
                for k in range(3):
                    cinA(s, k)
            cinB_pair(0)
            nc.tensor.matmul(ps[2][:], wpB[2][:], mB[:, 4:4 + TOK],
                             start=False, stop=False)
            nc.tensor.matmul(ps[1][:], wpB[1][:], mB[:, 4:4 + TOK],
                             start=False, stop=False)
            nc.tensor.matmul(ps[2][:], w3B[2][:], mB[0:64, 8:8 + TOK],
                             start=False, stop=True)
            nc.tensor.matmul(ps[1][:], w3B[1][:], mB[0:64, 8:8 + TOK],
                             start=False, stop=True)

            # sigmoids: S0 split in halves so rmemA can start sooner; pin the
            # queue order with explicit edges (the backend scheduler
            # otherwise hoists sig2 between the halves, delaying rmemA ~1us)
            sig = [act.tile([128, TOK], F32R, tag=f"sig{s}", name=f"sig{s}")
                   for s in range(3)]
            prev = None
            for h0, h1 in ((0, HTOK), (HTOK, TOK)):
                prev = nc.scalar.activation(sig[0][:, h0:h1], ps[0][:, h0:h1],
                                            AF.Sigmoid, bias=bias[0][:, 0:1])
            s2i = nc.scalar.activation(sig[2][:], ps[2][:], AF.Sigmoid,
                                       bias=bias[2][:, 0:1])
            add_dep_helper(s2i.ins, prev.ins, sync=False,
                           reason="sig0 halves before sig2")
            s1i = nc.scalar.activation(sig[1][:], ps[1][:], AF.Sigmoid,
                                       bias=bias[1][:, 0:1])
            add_dep_helper(s1i.ins, s2i.ins, sync=False,
                           reason="sig2 before sig1")

            # rmemA halves; fused B mul gives rmemB (rows 0:64) + uB (64:128)
            for h0, h1 in ((0, HTOK), (HTOK, TOK)):
                nc.vector.tensor_mul(rmemA[:, 4 + h0:4 + h1],
                                     sig[0][:, h0:h1], mA[:, 4 + h0:4 + h1])
            nc.vector.tensor_mul(sB[:, 4:4 + TOK], sig[2][:], mB[:, 4:4 + TOK])
            # u = gate * shifted (A half)
            uA = tmp.tile([128, TOK], F32R, tag="uA", name="uA")
            nc.vector.tensor_mul(uA[:], sig[1][:], mA[:, 0:TOK])

            # keep-warm dummies bridge the rg->cand rmemA latency
            dummy = psum.tile([128, 384], F32, tag="dm", name=f"dm{t}")
            for dk in range(N_DUMMY_MID):
                nc.tensor.matmul(dummy[:], wAn[0, 0][:],
                                 mA[:, 0:384], start=True, stop=True)

            # cand conv: A slot first (drives the critical tail); cinB taps
            # read sB[0:64] (rmemB) directly via column shifts
            psn = [psum.tile([128, TOK], F32, tag="nA", name="psnA"),
                   psum.tile([64, TOK], F32, tag="nB", name="psnB")]
            # order: cinA0(128c), cinB0(64c), cinB1(64c), cinA1(128c) - candA
            # still closes at matmul 6/12 but only 2 contraction switches
            for k in range(3):
                nc.tensor.matmul(psn[0][:], wAn[0, k][:],
                                 rmemA[:, k * 4: k * 4 + TOK],
                                 start=(k == 0), stop=False)
            for k in range(3):
                nc.tensor.matmul(psn[0][:], wBn[0, k][:],
                                 sB[0:64, k * 4: k * 4 + TOK],
                                 start=False, stop=(k == 2))
            for k in range(3):
                nc.tensor.matmul(psn[1][:], wBn[1, k][:],
                                 sB[0:64, k * 4: k * 4 + TOK],
                                 start=(k == 0), stop=False)
            for k in range(3):
                nc.tensor.matmul(psn[1][:], wAn[1, k][:],
                                 rmemA[:, k * 4: k * 4 + TOK],
                                 start=False, stop=(k == 2))

            # A-half tail split into halves: tanh -> q -> sub per 256 cols
            candA = act.tile([128, TOK], F32R, tag="cdA", name="cdA")
            candB = act.tile([128, TOK], F32R, tag="cdB", name="cdB")
            qA = tmp.tile([128, TOK], F32R, tag="qA", name="qA")
            qB = tmp.tile([128, TOK], F32R, tag="qB", name="qB")
            for h0, h1 in ((0, HTOK), (HTOK, TOK)):
                nc.scalar.activation(candA[:, h0:h1], psn[0][:, h0:h1], AF.Tanh)
            # tanhB crosses PSUM base 0 -> SBUF base 64
            nc.scalar.activation(candB[64:128, :], psn[1][:], AF.Tanh)
            for h0, h1 in ((0, HTOK), (HTOK, TOK)):
                nc.vector.scalar_tensor_tensor(
                    qA[:, h0:h1], sig[1][:, h0:h1], 1.0, candA[:, h0:h1],
                    op0=ALU.subtract, op1=ALU.mult)

            # keep-warm dummies for the restart gap: reading qA pins them
            # after the tail starts (dep-free dummies get hoisted by the
            # scheduler into the mid-step slot, delaying the cand conv)
            for dk in range(N_DUMMY_TAIL):
                nc.tensor.matmul(dummy[:], wAn[0, 0][:],
                                 qA[:, 0:384], start=True, stop=True)
            for h0, h1 in ((0, HTOK), (HTOK, TOK)):
                nc.vector.tensor_sub(nA[:, 4 + h0:4 + h1], uA[:, h0:h1],
                                     qA[:, h0:h1])
            # B-half combine at partition base 64; subB writes base 0
            nc.vector.scalar_tensor_tensor(
                qB[64:128, :], sig[2][64:128, :], 1.0, candB[64:128, :],
                op0=ALU.subtract, op1=ALU.mult)
            nc.vector.tensor_sub(nB[0:64, 4:4 + TOK], sB[64:128, 4:4 + TOK],
                                 qB[64:128, :])
            nc.vector.tensor_copy(nB[64:128, 8:8 + TOK], nB[0:64, 4:4 + TOK])

            cur = 1 - cur

        # --- output transform: mem[cur] -> out[b,l,c] -----------------------
        for b in range(BLOC):
            osb = tmp.tile([L, C], F32, tag="oload")
            for ci, (c0, cl) in enumerate(((0, 128), (128, 64))):
                ps = psum.tile([L, cl], F32R, tag=f"tp{ci}")
                nc.tensor.transpose(ps[:], mem[cur, ci][0:cl, 4 + b: 4 + b + 4 * L: 4],
                                    identr[0:cl, 0:cl])
                nc.vector.tensor_copy(osb[:, c0:c0 + cl], ps[:])
            nc.sync.dma_start(out_d[b], osb[:])

    nc.compile()
    return nc


_built = {}


def _get(steps=STEPS):
    if steps not in _built:
        _built[steps] = build(steps)
    return _built[steps]


def kernel(x, w_reset, b_reset, w_gate, b_gate, w_cand, b_cand, steps=STEPS,
           trace=False):
    nc = _get(steps)
    ident = np.eye(128, dtype=np.float32)
    base = {"w_reset": np.asarray(w_reset, np.float32),
            "b_reset": np.asarray(b_reset, np.float32),
            "w_gate": np.asarray(w_gate, np.float32),
            "b_gate": np.asarray(b_gate, np.float32),
            "w_cand": np.asarray(w_cand, np.float32),
            "b_cand": np.asarray(b_cand, np.float32),
            "ident": ident}
    x = np.asarray(x, np.float32)
    in_maps = [dict(base, x=np.ascontiguousarray(x[i * BLOC:(i + 1) * BLOC]))
               for i in range(NCORES)]
    res = run_bass_kernel_spmd(nc, in_maps, core_ids=list(range(NCORES)),
                               trace=trace)
    out = np.concatenate([res.results[i]["out"] for i in range(NCORES)], axis=0)
    if trace:
        return out, res
    return out


if __name__ == "__main__":
    rng = np.random.default_rng(0)
    scale = 1.0 / np.sqrt(3 * C)
    ins = {
        "x": rng.standard_normal((B, L, C), dtype=np.float32),
        "w_reset": (rng.standard_normal((3, C, C)) * scale).astype(np.float32),
        "b_reset": np.full(C, 0.5, np.float32),
        "w_gate": (rng.standard_normal((3, C, C)) * scale).astype(np.float32),
        "b_gate": np.full(C, 0.7, np.float32),
        "w_cand": (rng.standard_normal((3, C, C)) * scale).astype(np.float32),
        "b_cand": np.zeros(C, np.float32),
    }
    out = kernel(**ins, steps=2)
    print("smoke ok", out.shape, out.dtype)
